# revision 37
# baseline (speedup 1.0000x reference)
"""Segment-sum (scatter-add) kernel for Trainium2, 8 NeuronCores.

Strategy
--------
out[n, :] = sum_{e : index[e] == n} input[e, :]   (N=50000 segments, d=64)

Host side (data movement / re-encoding only, no arithmetic reduction):
  1. argsort(index) -> edges grouped by destination segment.
  2. Greedily pack *whole segments* (in id order) into fixed-capacity
     "chunks": each chunk covers <= 32 consecutive segment ids and
     <= 1024 edges (= 8 tiles x 128 edge rows).  Fill rate ~98%.
  3. Chunks are split contiguously across the 8 cores (each core owns a
     disjoint segment-id range -> no inter-core reduction needed).
  4. Edge rows are cast fp32 -> fp16 (rel err ~2^-11, far inside the
     2e-2 gate); this halves HBM traffic vs fp32 and runs the Tensor
     engine at 16-bit rate.
  5. Per core, edge rows are laid out partition-major so every DMA is a
     dense [128, W] strip.

Device side (all FLOPs): chunks are processed in PSUM groups of 8
(one PSUM bank); x is DMAed in ~2 MB blocks covering two groups
(l / iota are loaded first on the same sync HWDGE queue so the first
one-hot starts early).  Per group: the Vector engine builds a batched
one-hot [128 edges, 64 tiles x 32 segs] (iota == local_index, fp16,
16 oh buffers so DVE runs ~8 groups ahead of PE), then one fp16
matmul per tile psum[32, cc*64:...] += oh_t.T @ x_t accumulated over
each chunk's 8 tiles.  Flush: ScalarE cast-copies PSUM f32 -> SBUF
f16 into one of 4 partition bands of a [128, 512] block; every 4
groups the block is written out with a full-16-engine DMA.

Known limit: the fully unrolled PE stream (1600 LDWEIGHTS + 1600
MATMUL = 13 IRAM blocks) pays an ~2.5 us instruction-fetch stall per
16 KiB block boundary (~25 us total) which is the critical path at
~102 us; a For_i_pipelined hardware loop was tried and measured
slower (144 us) due to back-edge and dynamic-AP overheads.

Host finalization: place per-chunk row blocks into the [50000, 64]
output (pure scatter placement; np.add.at only if a segment ever had
to be split across chunks, which does not happen at these shapes).
"""

import os
import sys

for _p in ("/opt/trn_rl_repo", "/opt/pypackages"):
    if _p not in sys.path:
        sys.path.append(_p)

import numpy as np
import ml_dtypes

import concourse.mybir as mybir
from concourse import bacc
from concourse.mybir import AluOpType
from concourse.tile import TileContext
from concourse.bass_utils import run_bass_kernel_spmd

N_CORES = 8
P = 128               # partitions / contraction dim per tile
D = 64                # feature dim
SEGS_PER_CHUNK = 32   # one-hot width / psum partition dim
TILES_PER_CHUNK = 8
EDGES_PER_CHUNK = TILES_PER_CHUNK * P   # 1024
CHUNKS_PER_STRIP = 8  # per-core chunk count is padded to a multiple of this
MAX_STRIP_CHUNKS = int(os.environ.get("MAXSTRIP", "8"))  # chunks per x DMA / psum group
CHUNKS_PER_PSUM = 8   # chunks per PSUM tile (8 * 64 f32 = 512 = one bank)
BANDS = P // SEGS_PER_CHUNK  # output bands per packed block

F32 = mybir.dt.float32
F16 = mybir.dt.float16
NP_F16 = np.float16


# --------------------------------------------------------------------------
# host-side packing
# --------------------------------------------------------------------------

def pack_chunks(index: np.ndarray, n_segments: int):
    """Group sorted edges into fixed-capacity chunks of whole segments.

    Returns (order, chunk_seg_base, chunk_nseg, chunk_edge_start, chunk_nedge).
    """
    index = np.asarray(index).astype(np.int64, copy=False).ravel()
    order = np.argsort(index, kind="stable")
    counts = np.bincount(index, minlength=n_segments)

    seg_base, nsegs, edge_start, nedges = [], [], [], []
    s = 0
    epos = 0
    counts_list = counts.tolist()
    while s < n_segments:
        c = counts_list[s]
        if c > EDGES_PER_CHUNK:
            # split one oversized segment across several chunks
            left = c
            while left > 0:
                take = min(left, EDGES_PER_CHUNK)
                seg_base.append(s); nsegs.append(1)
                edge_start.append(epos); nedges.append(take)
                epos += take
                left -= take
            s += 1
            continue
        base = s
        tot = 0
        ns = 0
        while (
            s < n_segments
            and ns < SEGS_PER_CHUNK
            and tot + counts_list[s] <= EDGES_PER_CHUNK
        ):
            tot += counts_list[s]
            ns += 1
            s += 1
        seg_base.append(base); nsegs.append(ns)
        edge_start.append(epos); nedges.append(tot)
        epos += tot
    return (
        order,
        np.array(seg_base, dtype=np.int64),
        np.array(nsegs, dtype=np.int64),
        np.array(edge_start, dtype=np.int64),
        np.array(nedges, dtype=np.int64),
    )


def build_device_arrays(input_np, index_np, n_segments):
    """Returns (per_core, in_maps, assemble)."""
    input_np = np.asarray(input_np, dtype=np.float32).reshape(-1, D)
    index_np = np.asarray(index_np).astype(np.int64, copy=False).ravel()
    n_edges = input_np.shape[0]

    order, seg_base, nseg, e_start, ne = pack_chunks(index_np, n_segments)
    n_chunks = len(seg_base)
    # same chunk count on every core (SPMD), whole strips
    per_core = -(-n_chunks // N_CORES)
    per_core = -(-per_core // CHUNKS_PER_STRIP) * CHUNKS_PER_STRIP
    total_chunks = per_core * N_CORES

    # slot id for every edge (chunks are contiguous runs in sorted order)
    edge_chunk = np.repeat(np.arange(n_chunks), ne)
    within = np.arange(n_edges) - np.repeat(e_start, ne)
    slot = edge_chunk * EDGES_PER_CHUNK + within

    idx_sorted = index_np[order]
    local_row = (idx_sorted - seg_base[edge_chunk]).astype(np.float32)

    total_slots = total_chunks * EDGES_PER_CHUNK
    X_all = np.zeros((total_slots, D), dtype=NP_F16)
    X_all[slot] = input_np[order].astype(NP_F16)
    L_all = np.zeros(total_slots, dtype=NP_F16)
    L_all[slot] = local_row  # small ints, exact in fp16

    n_tiles_core = per_core * TILES_PER_CHUNK
    iota = np.broadcast_to(
        np.arange(SEGS_PER_CHUNK, dtype=NP_F16)[None, :], (P, SEGS_PER_CHUNK)
    ).copy()

    in_maps = []
    for c in range(N_CORES):
        lo_s = c * per_core * EDGES_PER_CHUNK
        hi_s = lo_s + per_core * EDGES_PER_CHUNK
        # per tile: [128 edges, 64 cols] fp16
        xt = X_all[lo_s:hi_s].reshape(n_tiles_core, P, D)
        xc = xt.transpose(1, 0, 2).reshape(P, n_tiles_core * D)
        lc = (
            L_all[lo_s:hi_s]
            .reshape(n_tiles_core, P)
            .transpose(1, 0)
        )
        in_maps.append(
            {
                "x": np.ascontiguousarray(xc),
                "l": np.ascontiguousarray(lc),
                "iota": iota,
            }
        )

    # group/block layout must match build_bass
    strips = build_strips(per_core)
    groups = []
    for c0, ncs in strips:
        r = ncs
        off = 0
        while r > 0:
            take = min(CHUNKS_PER_PSUM, r)
            groups.append((c0 + off, take))
            off += take
            r -= take
    n_blocks = -(-len(groups) // BANDS)
    band_idx = np.zeros(per_core, dtype=np.int64)
    blk_idx = np.zeros(per_core, dtype=np.int64)
    slot_idx = np.zeros(per_core, dtype=np.int64)
    for gi, (cb, gc) in enumerate(groups):
        for j in range(gc):
            band_idx[cb + j] = gi % BANDS
            blk_idx[cb + j] = gi // BANDS
            slot_idx[cb + j] = j

    def assemble(core_outs):
        # core_outs: list of [BANDS*SEGS_PER_CHUNK, n_blocks*CHUNKS_PER_PSUM*D]
        # -> [total_chunks * SEGS_PER_CHUNK, D] rows of (chunk, local_row)
        rows = np.concatenate(
            [
                np.asarray(o, dtype=np.float32)
                .reshape(BANDS, SEGS_PER_CHUNK, n_blocks, CHUNKS_PER_PSUM, D)[
                    band_idx, :, blk_idx, slot_idx, :
                ]
                .reshape(per_core * SEGS_PER_CHUNK, D)
                for o in core_outs
            ],
            axis=0,
        )
        row_seg = np.full(total_chunks * SEGS_PER_CHUNK, -1, dtype=np.int64)
        for i in range(n_chunks):
            row_seg[
                i * SEGS_PER_CHUNK : i * SEGS_PER_CHUNK + nseg[i]
            ] = np.arange(seg_base[i], seg_base[i] + nseg[i])
        valid = row_seg >= 0
        out = np.zeros((n_segments, D), dtype=np.float32)
        targets = row_seg[valid]
        vals = rows[valid]
        if len(np.unique(targets)) == len(targets):
            out[targets] = vals
        else:  # a segment was split across chunks
            np.add.at(out, targets, vals)
        return out

    return per_core, in_maps, assemble


# --------------------------------------------------------------------------
# device kernel
# --------------------------------------------------------------------------

def build_strips(n_chunks):
    """Strip sizes: small head ramp (compute starts early), max-size body,
    small tail ramp (short trailing compute after the last DMA byte)."""
    head = [int(v) for v in os.environ.get("RAMP", "2,4,8").split(",") if v]
    strips = []
    c = 0
    for take in head:
        if c + take <= n_chunks and n_chunks - (c + take) >= MAX_STRIP_CHUNKS:
            strips.append((c, take))
            c += take
    rem = n_chunks - c
    tail = [int(v) for v in os.environ.get("TAILRAMP", "8,4,2,2").split(",") if v]
    tail_sum = sum(tail)
    sizes = []
    while rem > MAX_STRIP_CHUNKS + tail_sum:
        sizes.append(MAX_STRIP_CHUNKS)
        rem -= MAX_STRIP_CHUNKS
    while rem > tail_sum:
        sizes.append(rem - tail_sum)
        rem = tail_sum
    for t in tail:
        if rem <= 0:
            break
        take = min(t, rem)
        sizes.append(take)
        rem -= take
    assert rem == 0 or sum(sizes) + c == n_chunks - rem
    while rem > 0:  # tiny n_chunks fallback
        take = min(2, rem)
        sizes.append(take)
        rem -= take
    for take in sizes:
        strips.append((c, take))
        c += take
    assert c == n_chunks
    return strips


def build_bass(n_chunks: int):
    nc = bacc.Bacc(
        "TRN2", target_bir_lowering=False, debug=False, num_devices=N_CORES
    )
    assert n_chunks % CHUNKS_PER_STRIP == 0
    n_tiles = n_chunks * TILES_PER_CHUNK
    max_strip_tiles = MAX_STRIP_CHUNKS * TILES_PER_CHUNK
    iota_w = max_strip_tiles * SEGS_PER_CHUNK

    group_tiles = CHUNKS_PER_PSUM * TILES_PER_CHUNK
    X = nc.dram_tensor("x", [P, n_tiles * D], F16, kind="ExternalInput")
    L = nc.dram_tensor("l", [P, n_tiles], F16, kind="ExternalInput")
    IOTA = nc.dram_tensor("iota", [P, SEGS_PER_CHUNK], F16, kind="ExternalInput")

    strips = build_strips(n_chunks)
    # global psum-group list: (chunk_base, n_chunks_in_group, strip_end_flag)
    groups = []
    for c0, ncs in strips:
        r = ncs
        off = 0
        while r > 0:
            take = min(CHUNKS_PER_PSUM, r)
            groups.append((c0 + off, take))
            off += take
            r -= take
    n_groups_total = len(groups)
    n_blocks = -(-n_groups_total // BANDS)
    # packed output: block b holds groups BANDS*b.. in partition bands of
    # SEGS_PER_CHUNK rows, each band up to CHUNKS_PER_PSUM chunks of 64 cols
    OUT = nc.dram_tensor(
        "out", [BANDS * SEGS_PER_CHUNK, n_blocks * CHUNKS_PER_PSUM * D],
        F16, kind="ExternalOutput",
    )

    group_tiles = CHUNKS_PER_PSUM * TILES_PER_CHUNK

    with TileContext(nc) as tc:
        with (
            tc.tile_pool(name="const", bufs=1) as cpool,
            tc.tile_pool(name="xin", bufs=4) as xpool,
            tc.tile_pool(name="oh", bufs=16) as ohpool,
            tc.tile_pool(name="acc", bufs=4, space="PSUM") as ppool,
            tc.tile_pool(name="outp", bufs=3) as opool,
        ):
            # l / iota first on the sync queue: they land before the x
            # stream loads HBM, so the first one-hot starts early
            iota_t = cpool.tile([P, SEGS_PER_CHUNK], F16)
            nc.sync.dma_start(out=iota_t[:], in_=IOTA[:, :])
            l_t = cpool.tile([P, n_tiles], F16)
            nc.sync.dma_start(out=l_t[:], in_=L[:, :])

            # x DMAs cover PAIRS of psum groups (~2 MB) for better DMA
            # efficiency; compute still proceeds per group
            pair_w = 2 * group_tiles * D
            xs = None
            xoff = 0
            for gi, (cb, gc) in enumerate(groups):
                band = gi % BANDS
                blk = gi // BANDS
                gt = gc * TILES_PER_CHUNK
                gt0 = cb * TILES_PER_CHUNK
                if gi % 2 == 0:
                    # tiles covered by this pair of groups (contiguous)
                    pt = gt + (
                        groups[gi + 1][1] * TILES_PER_CHUNK
                        if gi + 1 < len(groups)
                        else 0
                    )
                    xs = xpool.tile([P, pair_w], F16, tag="xs")
                    nc.sync.dma_start(
                        out=xs[:, : pt * D],
                        in_=X[:, gt0 * D : (gt0 + pt) * D],
                    )
                    xoff = 0
                # one-hot for this psum group: [128, tile, seg]
                oh = ohpool.tile([P, group_tiles * SEGS_PER_CHUNK], F16, tag="oh")
                lb = (
                    l_t[:, gt0 : gt0 + gt]
                    .unsqueeze(2)
                    .broadcast_to([P, gt, SEGS_PER_CHUNK])
                )
                ib = (
                    iota_t[:]
                    .unsqueeze(1)
                    .broadcast_to([P, gt, SEGS_PER_CHUNK])
                )
                nc.vector.tensor_tensor(
                    oh[:, : gt * SEGS_PER_CHUNK].rearrange(
                        "p (t g) -> p t g", t=gt, g=SEGS_PER_CHUNK
                    ),
                    ib,
                    lb,
                    AluOpType.is_equal,
                )
                ps = ppool.tile(
                    [SEGS_PER_CHUNK, CHUNKS_PER_PSUM * D], F32, tag="ps"
                )
                for cc in range(gc):
                    for t in range(TILES_PER_CHUNK):
                        ti = cc * TILES_PER_CHUNK + t
                        nc.tensor.matmul(
                            ps[:, cc * D : (cc + 1) * D],
                            lhsT=oh[:, ti * SEGS_PER_CHUNK : (ti + 1) * SEGS_PER_CHUNK],
                            rhs=xs[:, (xoff + ti) * D : (xoff + ti + 1) * D],
                            start=(t == 0),
                            stop=(t == TILES_PER_CHUNK - 1),
                        )
                # flush: cast-copy PSUM f32 -> SBUF f16 band of the packed
                # output block (BANDS groups of seg-rows -> 128 partitions)
                if band == 0:
                    ost = opool.tile(
                        [BANDS * SEGS_PER_CHUNK, CHUNKS_PER_PSUM * D], F16, tag="ost"
                    )
                ob = ost[
                    band * SEGS_PER_CHUNK : (band + 1) * SEGS_PER_CHUNK,
                    : gc * D,
                ]
                nc.scalar.copy(ob, ps[:, : gc * D])
                xoff += gt
                if band == BANDS - 1 or gi == n_groups_total - 1:
                    nc.scalar.dma_start(
                        out=OUT[
                            :, blk * CHUNKS_PER_PSUM * D : (blk + 1) * CHUNKS_PER_PSUM * D
                        ],
                        in_=ost[:],
                    )
    nc.compile()
    return nc


# --------------------------------------------------------------------------
# entry point
# --------------------------------------------------------------------------

def _run(input_np, index_np, n_segments, trace=False, trace_kwargs=None):
    per_core, in_maps, assemble = build_device_arrays(
        input_np, index_np, n_segments
    )
    nc = build_bass(per_core)
    res = run_bass_kernel_spmd(
        nc,
        in_maps,
        core_ids=list(range(N_CORES)),
        trace=trace,
        **(trace_kwargs or {}),
    )
    outs = [np.asarray(r["out"]) for r in res.results]
    return assemble(outs), res


def kernel(input, index):
    out, _ = _run(np.asarray(input), np.asarray(index), 50000)
    return out
